# revision 7
# baseline (speedup 1.0000x reference)
"""Trainium2 Bass kernel for:
    S = sigmoid(x[:,None,None,:] * w - q)      # [B, OUT, M, IN]
    A = tanh(m)                                # [OUT, 1, IN]
    D = sum(S * A, axis=3)                     # [B, OUT, M]
    O = sum(sigmoid(D), axis=2)                # [B, OUT]
with B=256, OUT=256, M=8, IN=512 (fp32 inputs).

Distribution: tensor-parallel over OUT across 8 NeuronCores (32 output
neurons per core); x is replicated.  No collectives needed — each core
computes its O[:, o_shard] slice and the host concatenates.

Per-core dataflow (i = IN index on partitions, 4 tiles of 128):
  DVE : t[i, om, b] = x[b, i] * w[om, i] - q[om, i]   (fused tensor_scalar,
        per-partition fp32 scalars, bf16 streams -> 4x mode)
  ACT : S = sigmoid(t)  in [128, 32*256] blocks (the critical path:
        1 elem/lane/cycle, so big free dims amortize the fixed overhead)
  PE  : D[o, mm, b] += A[o, i] . S[i, om, b]  via matmuls whose stationary
        weights are zero-padded [128, 32] tiles with tanh(m) in column
        o_local - places each output row at its PSUM partition while
        adding zero elsewhere.
  ACT : sigmoid(D) on the [32, 2048] PSUM accumulator
  DVE : reduce over mm (strided view) -> O^T shard [32, 256] -> DMA out.
"""

import sys

if "/opt/trn_rl_repo" not in sys.path:
    sys.path.insert(0, "/opt/trn_rl_repo")

import numpy as np

B, OUT, M, IN = 256, 256, 8, 512
NCORES = 8
O_PER_CORE = OUT // NCORES          # 32
OM_PER_CORE = O_PER_CORE * M        # 256
NIT = IN // 128                     # 4 partition tiles over IN
OM_BLK = 32                         # oms per (it, blk) group
NBLK = OM_PER_CORE // OM_BLK        # 8
N_AF = 6                            # oms per group computed fully on ACT
                                    # (fused sigmoid(w*x - q)); the rest go
                                    # DVE affine -> one big ACT sigmoid.
                                    # Chosen to balance DVE vs ACT busy time.

_CACHE = {}


def _build_nc():
    import concourse.bacc as bacc
    import concourse.bass as bass
    import concourse.mybir as mybir
    import concourse.tile as tile

    f32 = mybir.dt.float32
    bf16 = mybir.dt.bfloat16
    Act = mybir.ActivationFunctionType
    Alu = mybir.AluOpType

    nc = bacc.Bacc("TRN2", target_bir_lowering=False, debug=False)

    xT_d = nc.dram_tensor("xT", [128, NIT, B], bf16, kind="ExternalInput")
    wT_d = nc.dram_tensor("wT", [128, NIT, OM_PER_CORE], f32, kind="ExternalInput")
    # staged NEGATED: bias/addend is -q for both the DVE and ACT paths
    qT_d = nc.dram_tensor("qT", [128, NIT, OM_PER_CORE], f32, kind="ExternalInput")
    mT_d = nc.dram_tensor("mT", [128, NIT * O_PER_CORE], f32, kind="ExternalInput")
    out_d = nc.dram_tensor("out", [O_PER_CORE, B], f32, kind="ExternalOutput")

    with tile.TileContext(nc) as tc:
        with (
            tc.tile_pool(name="consts", bufs=1) as consts,
            tc.tile_pool(name="tpool", bufs=3) as tpool,
            tc.tile_pool(name="spool", bufs=3) as spool,
            tc.tile_pool(name="psum", bufs=1, space="PSUM") as psum,
            tc.tile_pool(name="epi", bufs=1) as epi,
        ):
            xT = consts.tile([128, NIT, B], bf16)
            wT = consts.tile([128, NIT, OM_PER_CORE], f32)
            qT = consts.tile([128, NIT, OM_PER_CORE], f32)
            mT = consts.tile([128, NIT * O_PER_CORE], f32)
            a16 = consts.tile([128, NIT * O_PER_CORE], bf16)
            # zero-padded stationary weights: block (it, o) holds tanh(m)
            # for (o, i-tile it) in column o, zeros elsewhere
            apad = consts.tile([128, NIT * O_PER_CORE, O_PER_CORE], bf16)

            nc.sync.dma_start(out=xT, in_=xT_d.ap())
            nc.sync.dma_start(out=wT, in_=wT_d.ap())
            nc.sync.dma_start(out=qT, in_=qT_d.ap())
            nc.sync.dma_start(out=mT, in_=mT_d.ap())

            nc.scalar.activation(a16, mT, Act.Tanh)
            apad_flat = apad.rearrange("p a b -> p (a b)")
            nc.gpsimd.memset(apad_flat, 0.0)
            blk_w = O_PER_CORE  # 32 columns per (it, o) block
            for it in range(NIT):
                # diagonal strided view: col (it*32+o)*32 + o for o in 0..31
                base = apad_flat[:, it * blk_w * blk_w : (it + 1) * blk_w * blk_w]
                diag = bass.AP(
                    tensor=base.tensor,
                    offset=base.offset,
                    ap=[base.ap[0], [blk_w + 1, blk_w]],
                )
                nc.vector.tensor_copy(diag, a16[:, it * blk_w : (it + 1) * blk_w])

            dps = psum.tile([O_PER_CORE, M * B], f32)

            n_dve = OM_BLK - N_AF
            for it in range(NIT):
                for blk in range(NBLK):
                    s = spool.tile([128, OM_BLK, B], bf16)
                    t = tpool.tile([128, n_dve, B], bf16)
                    for j in range(n_dve):
                        om = blk * OM_BLK + j
                        nc.vector.tensor_scalar(
                            t[:, j, :],
                            xT[:, it, :],
                            wT[:, it, om : om + 1],
                            qT[:, it, om : om + 1],
                            Alu.mult,
                            Alu.add,
                        )
                    for j in range(n_dve, OM_BLK):
                        om = blk * OM_BLK + j
                        nc.scalar.activation(
                            s[:, j, :],
                            xT[:, it, :],
                            Act.Sigmoid,
                            bias=qT[:, it, om : om + 1],
                            scale=wT[:, it, om : om + 1],
                        )
                    nc.scalar.activation(s[:, :n_dve, :], t, Act.Sigmoid)
                    for o4 in range(OM_BLK // M):
                        o_loc = blk * (OM_BLK // M) + o4
                        lhsT = apad[:, it * O_PER_CORE + o_loc, :]
                        for p4 in range(4):
                            rhs = s[:, o4 * M + 2 * p4 : o4 * M + 2 * p4 + 2, :]
                            outp = dps[:, p4 * 512 : (p4 + 1) * 512]
                            first = it == 0 and blk == 0 and o4 == 0
                            last = it == NIT - 1 and blk == NBLK - 1 and o4 == 3
                            nc.tensor.matmul(
                                outp,
                                lhsT,
                                rhs,
                                start=first,
                                stop=last,
                                skip_group_check=True,
                            )

            dsig = epi.tile([O_PER_CORE, M * B], f32)
            nc.scalar.activation(dsig, dps, Act.Sigmoid)
            osb = epi.tile([O_PER_CORE, B], f32)
            # sum over mm: view [32, b, mm] with mm innermost (stride 256)
            dv = dsig.rearrange("p (mm b) -> p b mm", mm=M)
            nc.vector.tensor_reduce(osb, dv, mybir.AxisListType.X, Alu.add)
            nc.sync.dma_start(out=out_d.ap(), in_=osb)

    nc.compile()
    return nc


def _get_nc():
    if "nc" not in _CACHE:
        _CACHE["nc"] = _build_nc()
    return _CACHE["nc"]


def _prep_in_maps(x, w, q, m):
    import ml_dtypes

    x = np.asarray(x, np.float32)
    w = np.asarray(w, np.float32)
    q = np.asarray(q, np.float32)
    m = np.asarray(m, np.float32)

    # x^T tiled: xT[p, it, b] = x[b, it*128+p]
    xt = np.ascontiguousarray(
        x.T.reshape(NIT, 128, B).transpose(1, 0, 2)
    ).astype(ml_dtypes.bfloat16)

    in_maps = []
    for c in range(NCORES):
        o0 = c * O_PER_CORE
        ws = w[o0 : o0 + O_PER_CORE].reshape(OM_PER_CORE, IN)
        qs = -q[o0 : o0 + O_PER_CORE].reshape(OM_PER_CORE, IN)
        ms = m[o0 : o0 + O_PER_CORE, 0, :]  # [32, 512]
        wt = np.ascontiguousarray(ws.T.reshape(NIT, 128, OM_PER_CORE).transpose(1, 0, 2))
        qt = np.ascontiguousarray(qs.T.reshape(NIT, 128, OM_PER_CORE).transpose(1, 0, 2))
        mt = np.ascontiguousarray(
            ms.T.reshape(NIT, 128, O_PER_CORE).transpose(1, 0, 2)
        ).reshape(128, NIT * O_PER_CORE)
        in_maps.append({"xT": xt, "wT": wt, "qT": qt, "mT": mt})
    return in_maps


def kernel(x, w, q, m):
    from concourse import bass_utils

    nc = _get_nc()
    in_maps = _prep_in_maps(x, w, q, m)
    res = bass_utils.run_bass_kernel_spmd(
        nc, in_maps, core_ids=list(range(NCORES)), trace=False
    )
    parts = [res.results[c]["out"] for c in range(NCORES)]  # each [32, 256] = O^T shard
    return np.ascontiguousarray(np.concatenate(parts, axis=0).T.astype(np.float32))
